# revision 7
# baseline (speedup 1.0000x reference)
"""Trainium2 Bass kernel for nn_BPMOE (moe_routing).

Strategy: pure data-parallel over 8 cores. Each third of the batch
(src / pos-dst / neg-dst) is sharded identically so the edge-predictor
pairing stays core-local. Per core: B-on-partitions layout, tiles of
128 rows, groups of G tiles share DMA + batched routing math.

Host folds the gating weights:
  cat @ W = mem@(W1+W4) + S@((W2+W4)/4) + R@((W3+W4)/4)
            + x2'@(W5/16) + node@W6
with S = sum_e spa_e, R = sum_e relu(rec_e), x2' = mem*S*R.
Loss statistics (importance/load partial sums) come back per-core and
are reduced on host (the "all-reduce" of the sharding hint).
"""

import numpy as np
from contextlib import ExitStack

B_FULL = 90000
D = 100
NUM_EDGE = 30000
M_CORES = 8
THIRD = 3840              # padded rows per third per core (30 tiles)
BC = 3 * THIRD            # 11520 rows per core
NT = BC // 128            # 90 tiles
T3 = NT // 3              # 30 tiles per third
G = 10                    # tiles per group
NG = NT // G


def _make_body(BC, G):
    import concourse.bass as bass
    import concourse.tile as tile
    from concourse import mybir

    f32 = mybir.dt.float32
    Alu = mybir.AluOpType
    Act = mybir.ActivationFunctionType
    AX = mybir.AxisListType.X

    NT = BC // 128
    T3 = NT // 3
    NG = NT // G
    JE = G * 9

    def body(tc, outs, ins):
        nc = tc.nc
        x_mem, x_rec, x_spa, x_node, x_noi, x_msc, w_cat, w_pred, bcasts = ins
        o_pn, o_stats = outs
        ctx = tc.ctx if hasattr(tc, "ctx") else None

        with ExitStack() as ex:
            pool_const = ex.enter_context(tc.tile_pool(name="const", bufs=1))
            pool_in = ex.enter_context(tc.tile_pool(name="in", bufs=2))
            pool_f = ex.enter_context(tc.tile_pool(name="f", bufs=4))
            pool_ct = ex.enter_context(tc.tile_pool(name="ct", bufs=4))
            pool_rt = ex.enter_context(tc.tile_pool(name="rt", bufs=2))
            pool_b = ex.enter_context(tc.tile_pool(name="b", bufs=3))
            pool_acc = ex.enter_context(tc.tile_pool(name="acc", bufs=1))
            pool_pair = ex.enter_context(tc.tile_pool(name="pair", bufs=2))
            ps_t = ex.enter_context(tc.tile_pool(name="pst", bufs=2, space="PSUM"))
            ps_g = ex.enter_context(tc.tile_pool(name="psg", bufs=2, space="PSUM"))
            ps_o = ex.enter_context(tc.tile_pool(name="pso", bufs=1, space="PSUM"))
            ps_h = ex.enter_context(tc.tile_pool(name="psh", bufs=1, space="PSUM"))

            const = pool_const.tile([128, 631], f32)
            nc.sync.dma_start(const[:], bcasts)
            wc = pool_const.tile([100, 90], f32)
            nc.sync.dma_start(wc[:], w_cat.rearrange("(s p) c -> p s c", p=100))
            wp = pool_const.tile([100, 200], f32)
            nc.sync.dma_start(wp[:], w_pred)

            ident = const[:, 0:128]
            c0b = const[:, 128:228]
            c1b = const[:, 228:328]
            owb = const[:, 328:428]
            sbb = const[:, 428:528]
            dbb = const[:, 528:628]
            outbc = const[:, 628:629]
            one_b = const[:, 629:630]
            erfb = const[:, 630:631]

            imp = pool_acc.tile([128, 9], f32)
            nc.vector.memset(imp[:], 0.0)
            lod = pool_acc.tile([128, 9], f32)
            nc.vector.memset(lod[:], 0.0)
            h_all = pool_acc.tile([128, NT * 100], f32)

            def v9(ap):  # [128, JE] -> [128, G, 9]
                return ap.rearrange("p (j e) -> p j e", e=9)

            def jb(ap):  # [128, G] -> [128, G, 9] broadcast
                return ap.rearrange("p (j o) -> p j o", o=1).to_broadcast(
                    (128, G, 9)
                )

            for g in range(NG):
                r0, r1 = g * G * 128, (g + 1) * G * 128
                tmem = pool_in.tile([128, G * 100], f32, tag="tmem")
                nc.sync.dma_start(
                    tmem[:], x_mem[r0:r1].rearrange("(j p) d -> p j d", p=128)
                )
                trec = pool_in.tile([128, G * 400], f32, tag="trec")
                nc.sync.dma_start(
                    trec[:], x_rec[r0:r1].rearrange("(j p) d -> p j d", p=128)
                )
                tspa = pool_in.tile([128, G * 400], f32, tag="tspa")
                nc.sync.dma_start(
                    tspa[:], x_spa[r0:r1].rearrange("(j p) d -> p j d", p=128)
                )
                tnod = pool_in.tile([128, G * 100], f32, tag="tnod")
                nc.sync.dma_start(
                    tnod[:], x_node[r0:r1].rearrange("(j p) d -> p j d", p=128)
                )
                tnoi = pool_in.tile([128, G * 9], f32, tag="tnoi")
                nc.sync.dma_start(
                    tnoi[:], x_noi[r0:r1].rearrange("(j p) d -> p j d", p=128)
                )
                tmsc = pool_in.tile([128, G * 2], f32, tag="tmsc")
                nc.sync.dma_start(
                    tmsc[:], x_msc[r0:r1].rearrange("(j p) d -> p j d", p=128)
                )

                nc.scalar.activation(trec[:], trec[:], Act.Relu)
                nc.scalar.activation(tmem[:], tmem[:], Act.Relu)

                pg = ps_g.tile([128, G * 18], f32)

                for j in range(G):
                    t = g * G + j
                    mem = tmem[:, j * 100 : (j + 1) * 100]
                    rec4 = trec[:, j * 400 : (j + 1) * 400]
                    spa4 = tspa[:, j * 400 : (j + 1) * 400]
                    node = tnod[:, j * 100 : (j + 1) * 100]
                    ldg = tmsc[:, j * 2 : j * 2 + 1]

                    fac = pool_f.tile([128, 100], f32, tag="fac")
                    nc.vector.scalar_tensor_tensor(
                        fac[:], c1b, ldg, c0b, Alu.mult, Alu.add
                    )
                    # spa *= factor (broadcast over the 4 experts), in place
                    sp3 = spa4.rearrange("p (e d) -> p e d", e=4)
                    facb = fac[:].rearrange("p (o d) -> p o d", o=1).to_broadcast(
                        (128, 4, 100)
                    )
                    nc.gpsimd.tensor_tensor(sp3, sp3, facb, Alu.mult)

                    psa = ps_t.tile([128, 640], f32)
                    nc.tensor.matmul(
                        psa[0:100, 0:128], mem, ident,
                        is_transpose=True, start=True, stop=True,
                        skip_group_check=True,
                    )
                    for e in range(4):
                        nc.tensor.matmul(
                            psa[0:100, 128:256],
                            spa4[:, e * 100 : (e + 1) * 100], ident,
                            is_transpose=True, start=(e == 0), stop=(e == 3),
                            skip_group_check=True,
                        )
                    for e in range(4):
                        nc.tensor.matmul(
                            psa[0:100, 256:384],
                            rec4[:, e * 100 : (e + 1) * 100], ident,
                            is_transpose=True, start=(e == 0), stop=(e == 3),
                            skip_group_check=True,
                        )
                    nc.tensor.matmul(
                        psa[0:100, 512:640], node, ident,
                        is_transpose=True, start=True, stop=True,
                        skip_group_check=True,
                    )
                    catT = pool_ct.tile([100, 640], f32)
                    nc.scalar.copy(catT[:, 0:384], psa[0:100, 0:384])
                    nc.scalar.copy(catT[:, 512:640], psa[0:100, 512:640])
                    # x2'^T = mem^T * S^T * R^T, written straight into catT
                    x2t = pool_f.tile([100, 128], f32, tag="x2t")
                    nc.vector.tensor_mul(x2t[:], catT[:, 0:128], catT[:, 128:256])
                    nc.vector.tensor_mul(catT[:, 384:512], x2t[:], catT[:, 256:384])
                    for s in range(5):
                        nc.tensor.matmul(
                            pg[:, j * 18 : (j + 1) * 18],
                            catT[:, s * 128 : (s + 1) * 128],
                            wc[:, s * 18 : (s + 1) * 18],
                            start=(s == 0),
                            stop=(s == 4),
                            skip_group_check=True,
                        )

                # ---------- routing (batched over the group) ----------
                pg3 = pg[:].rearrange("p (j c) -> p j c", c=18)
                clean = pg3[:, :, 0:9]
                nzl = pg3[:, :, 9:18]

                stdt = pool_rt.tile([128, JE], f32, tag="stdt")
                nc.scalar.activation(v9(stdt[:]), nzl, Act.Exp)
                nc.scalar.activation(stdt[:], stdt[:], Act.Ln, bias=one_b)
                nc.vector.tensor_scalar_add(stdt[:], stdt[:], 0.01)

                nzy = pool_rt.tile([128, JE], f32, tag="nzy")
                nc.vector.tensor_mul(nzy[:], tnoi[:], stdt[:])
                nc.vector.tensor_tensor(v9(nzy[:]), v9(nzy[:]), clean, Alu.add)

                m1 = pool_rt.tile([128, G], f32, tag="m1")
                nc.vector.reduce_max(m1[:], v9(nzy[:]), axis=AX)
                mk1 = pool_rt.tile([128, JE], f32, tag="mk1")
                nc.vector.tensor_tensor(
                    v9(mk1[:]), v9(nzy[:]), jb(m1[:]), Alu.is_equal
                )
                ny2 = pool_rt.tile([128, JE], f32, tag="ny2")
                nc.vector.scalar_tensor_tensor(
                    ny2[:], mk1[:], -1e30, nzy[:], Alu.mult, Alu.add
                )
                m2 = pool_rt.tile([128, G], f32, tag="m2")
                nc.vector.reduce_max(m2[:], v9(ny2[:]), axis=AX)
                mk2 = pool_rt.tile([128, JE], f32, tag="mk2")
                nc.vector.tensor_tensor(
                    v9(mk2[:]), v9(ny2[:]), jb(m2[:]), Alu.is_equal
                )
                ny3 = pool_rt.tile([128, JE], f32, tag="ny3")
                nc.vector.scalar_tensor_tensor(
                    ny3[:], mk2[:], -1e30, ny2[:], Alu.mult, Alu.add
                )
                m3 = pool_rt.tile([128, G], f32, tag="m3")
                nc.vector.reduce_max(m3[:], v9(ny3[:]), axis=AX)

                dl = pool_rt.tile([128, G], f32, tag="dl")
                nc.vector.tensor_sub(dl[:], m1[:], m2[:])
                ed = pool_rt.tile([128, G], f32, tag="ed")
                nc.scalar.activation(ed[:], dl[:], Act.Exp, scale=-1.0)
                nc.vector.tensor_scalar_add(ed[:], ed[:], 1.0)
                g1 = pool_rt.tile([128, G], f32, tag="g1")
                nc.vector.reciprocal(g1[:], ed[:])
                g2 = pool_rt.tile([128, G], f32, tag="g2")
                nc.vector.tensor_scalar(g2[:], g1[:], -1.0, 1.0, Alu.mult, Alu.add)

                maskb = (
                    tmsc[:]
                    .rearrange("p (j c) -> p j c", c=2)[:, :, 1:2]
                    .to_broadcast((128, G, 9))
                )
                gt = pool_rt.tile([128, JE], f32, tag="gt")
                nc.vector.tensor_tensor(v9(gt[:]), v9(mk1[:]), jb(g1[:]), Alu.mult)
                gt2 = pool_rt.tile([128, JE], f32, tag="gt2")
                nc.vector.tensor_tensor(v9(gt2[:]), v9(mk2[:]), jb(g2[:]), Alu.mult)
                nc.vector.tensor_add(gt[:], gt[:], gt2[:])
                nc.vector.tensor_tensor(v9(gt[:]), v9(gt[:]), maskb, Alu.mult)

                t9 = pool_rt.tile([128, 9], f32, tag="t9")
                nc.vector.reduce_sum(
                    t9[:], gt[:].rearrange("p (j e) -> p e j", e=9), axis=AX
                )
                nc.vector.tensor_add(imp[:], imp[:], t9[:])

                isin = pool_rt.tile([128, JE], f32, tag="isin")
                nc.vector.tensor_tensor(
                    v9(isin[:]), v9(nzy[:]), jb(m3[:]), Alu.is_gt
                )
                d32 = pool_rt.tile([128, G], f32, tag="d32")
                nc.vector.tensor_sub(d32[:], m3[:], m2[:])
                thr = pool_rt.tile([128, JE], f32, tag="thr")
                nc.vector.tensor_tensor(v9(thr[:]), v9(isin[:]), jb(d32[:]), Alu.mult)
                nc.vector.tensor_tensor(v9(thr[:]), v9(thr[:]), jb(m2[:]), Alu.add)
                zz = pool_rt.tile([128, JE], f32, tag="zz")
                nc.vector.tensor_tensor(v9(zz[:]), clean, v9(thr[:]), Alu.subtract)
                rsd = pool_rt.tile([128, JE], f32, tag="rsd")
                nc.scalar.activation(rsd[:], stdt[:], Act.Ln)
                nc.scalar.activation(rsd[:], rsd[:], Act.Exp, scale=-1.0)
                nc.vector.tensor_mul(zz[:], zz[:], rsd[:])
                nc.scalar.activation(
                    zz[:], zz[:], Act.Erf,
                    scale=0.7071067811865476, bias=erfb,
                )
                nc.vector.tensor_scalar(zz[:], zz[:], 0.5, 0.5, Alu.mult, Alu.add)
                nc.vector.tensor_tensor(v9(zz[:]), v9(zz[:]), maskb, Alu.mult)
                t9b = pool_rt.tile([128, 9], f32, tag="t9b")
                nc.vector.reduce_sum(
                    t9b[:], zz[:].rearrange("p (j e) -> p e j", e=9), axis=AX
                )
                nc.vector.tensor_add(lod[:], lod[:], t9b[:])

                # ---------- combine + predictor ----------
                for j in range(G):
                    t = g * G + j
                    mem = tmem[:, j * 100 : (j + 1) * 100]
                    rec4 = trec[:, j * 400 : (j + 1) * 400]
                    spa4 = tspa[:, j * 400 : (j + 1) * 400]  # already *factor
                    tall = pool_b.tile([128, 900], f32, tag="tall")
                    for e in range(9):
                        gcol = gt[:, 9 * j + e : 9 * j + e + 1]
                        if e == 0:
                            src = mem
                        elif e < 5:
                            src = spa4[:, (e - 1) * 100 : e * 100]
                        else:
                            src = rec4[:, (e - 5) * 100 : (e - 4) * 100]
                        eng = nc.vector if e < 4 else nc.gpsimd
                        eng.tensor_scalar(
                            tall[:, e * 100 : (e + 1) * 100], src, gcol, None,
                            Alu.mult,
                        )
                    comb = pool_b.tile([128, 100], f32, tag="comb")
                    nc.vector.reduce_sum(
                        comb[:], tall[:].rearrange("p (e d) -> p d e", e=9),
                        axis=AX,
                    )
                    pso = ps_o.tile([128, 128], f32)
                    nc.tensor.matmul(
                        pso[0:100, 0:128], comb[:], ident,
                        is_transpose=True, start=True, stop=True,
                        skip_group_check=True,
                    )
                    outT = pool_b.tile([100, 128], f32, tag="outT")
                    nc.scalar.copy(outT[:], pso[0:100, :])
                    ph = ps_h.tile([128, 100], f32)
                    c = 0 if t < T3 else 1
                    nc.tensor.matmul(
                        ph[:], outT[:], wp[:, c * 100 : (c + 1) * 100],
                        start=True, stop=True, skip_group_check=True,
                    )
                    bias = sbb if t < T3 else dbb
                    nc.vector.tensor_add(
                        h_all[:, t * 100 : (t + 1) * 100], ph[:], bias
                    )

            # ---------- edge predictor pairing ----------
            W3 = T3 * 100
            o_pn_t = pool_acc.tile([128, 2 * T3], f32)
            owb3 = owb.rearrange("p (o d) -> p o d", o=1).to_broadcast(
                (128, T3, 100)
            )
            for half in range(2):
                off = (1 + half) * W3
                u = pool_pair.tile([128, W3], f32, tag="u")
                nc.vector.tensor_add(u[:], h_all[:, 0:W3], h_all[:, off : off + W3])
                nc.scalar.activation(u[:], u[:], Act.Relu)
                uv = u[:].rearrange("p (t d) -> p t d", d=100)
                nc.vector.tensor_tensor(uv, uv, owb3, Alu.mult)
                pc = pool_pair.tile([128, T3], f32, tag="pc")
                nc.vector.reduce_sum(pc[:], uv, axis=AX)
                nc.vector.tensor_scalar(
                    o_pn_t[:, half * T3 : (half + 1) * T3], pc[:], outbc, None,
                    Alu.add,
                )
            nc.sync.dma_start(o_pn, o_pn_t[:])

            stats_t = pool_acc.tile([128, 18], f32)
            nc.vector.tensor_copy(stats_t[:, 0:9], imp[:])
            nc.vector.tensor_copy(stats_t[:, 9:18], lod[:])
            nc.sync.dma_start(o_stats, stats_t[:])

    return body


def _host_prep(inputs):
    """Fold weights, shard + pad per core. Returns (ins_list, output_like)."""
    f32 = np.float32
    mem = np.ascontiguousarray(inputs["memory_feats"], f32)
    rec = np.ascontiguousarray(inputs["recent_raw"], f32).reshape(B_FULL, 400)
    spa = np.ascontiguousarray(inputs["spatial_raw"], f32).reshape(B_FULL, 400)
    node = np.ascontiguousarray(inputs["node_feats_src"], f32)
    noise = np.ascontiguousarray(inputs["noise"], f32)
    deg = inputs["degree"].astype(f32)
    logdeg = np.log(deg + 1.0).astype(f32)

    wg = np.asarray(inputs["w_gate"], f32)
    wn = np.asarray(inputs["w_noise"], f32)

    def fold(w):
        W1, W2, W3_, W4, W5, W6 = (w[i * 100 : (i + 1) * 100] for i in range(6))
        return np.concatenate(
            [W1 + W4, (W2 + W4) / 4.0, (W3_ + W4) / 4.0, W5 / 16.0, W6], axis=0
        )

    w_cat = np.concatenate([fold(wg), fold(wn)], axis=1).astype(f32)  # [500,18]
    w_pred = np.concatenate(
        [np.asarray(inputs["src_w"], f32), np.asarray(inputs["dst_w"], f32)],
        axis=1,
    )  # [100, 200]

    c0 = np.asarray(inputs["deg_coef"], f32)[0, :, 0]
    c1 = np.asarray(inputs["deg_coef"], f32)[0, :, 1]
    ow = np.asarray(inputs["out_w"], f32)[:, 0]
    sb = np.asarray(inputs["src_b"], f32)
    db = np.asarray(inputs["dst_b"], f32)
    ob = float(np.asarray(inputs["out_b"], f32)[0])
    bcasts = np.zeros((128, 631), f32)
    bcasts[:, 629] = 1.0
    bcasts[:, 630] = -0.07071067811865476
    bcasts[:, 0:128] = np.eye(128, dtype=f32)
    bcasts[:, 128:228] = c0
    bcasts[:, 228:328] = c1
    bcasts[:, 328:428] = ow
    bcasts[:, 428:528] = sb
    bcasts[:, 528:628] = db
    bcasts[:, 628] = ob

    ins_list = []
    for k in range(M_CORES):
        xm = np.zeros((BC, 100), f32)
        xr = np.zeros((BC, 400), f32)
        xs = np.zeros((BC, 400), f32)
        xn = np.zeros((BC, 100), f32)
        xz = np.zeros((BC, 9), f32)
        xc = np.zeros((BC, 2), f32)
        for third in range(3):
            lo = third * NUM_EDGE + k * THIRD
            hi = min(lo + THIRD, (third + 1) * NUM_EDGE)
            n = hi - lo
            if n <= 0:
                continue
            dst = slice(third * THIRD, third * THIRD + n)
            xm[dst] = mem[lo:hi]
            xr[dst] = rec[lo:hi]
            xs[dst] = spa[lo:hi]
            xn[dst] = node[lo:hi]
            xz[dst] = noise[lo:hi]
            xc[dst, 0] = logdeg[lo:hi]
            xc[dst, 1] = 1.0
        ins_list.append((xm, xr, xs, xn, xz, xc, w_cat, w_pred, bcasts))

    output_like = (
        np.zeros((128, 2 * T3), f32),
        np.zeros((128, 18), f32),
    )
    return ins_list, output_like


def kernel(**inputs):
    assert int(inputs["neg_samples"]) == 1
    from concourse import tile
    from concourse.bass_test_utils import run_kernel

    ins_list, output_like = _host_prep(inputs)
    body = _make_body(BC, G)
    res = run_kernel(
        body,
        None,
        ins_list,
        output_like=[output_like] * M_CORES,
        bass_type=tile.TileContext,
        check_with_sim=False,
        check_with_hw=True,
        num_cores=M_CORES,
    )
    assert res is not None

    pos = np.zeros((NUM_EDGE, 1), np.float32)
    neg = np.zeros((NUM_EDGE, 1), np.float32)
    imp = np.zeros(9, np.float64)
    lod = np.zeros(9, np.float64)
    for k in range(M_CORES):
        rk = res.results[k]
        o_pn = next(v for v in rk.values() if v.shape == (128, 2 * T3))
        o_st = next(v for v in rk.values() if v.shape == (128, 18))
        n = min(THIRD, NUM_EDGE - k * THIRD)
        if n > 0:
            # row (t*128+p) of this core's third -> o_pn[p, t]
            posk = o_pn[:, 0:T3].T.reshape(-1)[:n]   # [t, p] -> flat
            negk = o_pn[:, T3 : 2 * T3].T.reshape(-1)[:n]
            pos[k * THIRD : k * THIRD + n, 0] = posk
            neg[k * THIRD : k * THIRD + n, 0] = negk
        imp += o_st[:, 0:9].sum(axis=0, dtype=np.float64)
        lod += o_st[:, 9:18].sum(axis=0, dtype=np.float64)

    def cv2(x):
        x = x.astype(np.float32)
        return np.var(x, ddof=1) / (np.mean(x) ** 2 + 1e-10)

    loss = np.float32(0.4 * (cv2(imp) + cv2(lod)))
    if getattr(res, "exec_time_ns", None):
        print(f"HW exec time: {res.exec_time_ns} ns")
    return pos, neg, loss


# revision 11
# speedup vs baseline: 1.0161x; 1.0161x over previous
"""Trainium2 Bass kernel for nn_BPMOE (moe_routing).

Strategy: pure data-parallel over 8 cores. Each third of the batch
(src / pos-dst / neg-dst) is sharded identically so the edge-predictor
pairing stays core-local. Per core: B-on-partitions layout, tiles of
128 rows, groups of G tiles share DMA + batched routing math.

Host folds the gating weights:
  cat @ W = mem@(W1+W4) + S@((W2+W4)/4) + R@((W3+W4)/4)
            + x2'@(W5/16) + node@W6
with S = sum_e spa_e, R = sum_e relu(rec_e), x2' = mem*S*R.
Loss statistics (importance/load partial sums) come back per-core and
are reduced on host (the "all-reduce" of the sharding hint).
"""

import numpy as np
from contextlib import ExitStack

B_FULL = 90000
D = 100
NUM_EDGE = 30000
M_CORES = 8
THIRD = 3840              # padded rows per third per core (30 tiles)
BC = 3 * THIRD            # 11520 rows per core
NT = BC // 128            # 90 tiles
T3 = NT // 3              # 30 tiles per third
G = 10                    # tiles per group
NG = NT // G


def _make_body(BC, G):
    import concourse.bass as bass
    import concourse.tile as tile
    from concourse import mybir

    f32 = mybir.dt.float32
    Alu = mybir.AluOpType
    Act = mybir.ActivationFunctionType
    AX = mybir.AxisListType.X

    NT = BC // 128
    T3 = NT // 3
    NG = NT // G
    JE = G * 9

    def body(tc, outs, ins):
        nc = tc.nc
        x_mem, x_rec, x_spa, x_node, x_noi, x_msc, w_cat, w_pred, bcasts = ins
        o_pn, o_stats = outs
        ctx = tc.ctx if hasattr(tc, "ctx") else None

        with ExitStack() as ex:
            pool_const = ex.enter_context(tc.tile_pool(name="const", bufs=1))
            pool_in = ex.enter_context(tc.tile_pool(name="in", bufs=2))
            pool_f = ex.enter_context(tc.tile_pool(name="f", bufs=4))
            pool_ct = ex.enter_context(tc.tile_pool(name="ct", bufs=4))
            pool_rt = ex.enter_context(tc.tile_pool(name="rt", bufs=3))
            pool_b = ex.enter_context(tc.tile_pool(name="b", bufs=4))
            pool_acc = ex.enter_context(tc.tile_pool(name="acc", bufs=1))
            pool_pair = ex.enter_context(tc.tile_pool(name="pair", bufs=2))
            ps_t = ex.enter_context(tc.tile_pool(name="pst", bufs=2, space="PSUM"))
            ps_g = ex.enter_context(tc.tile_pool(name="psg", bufs=2, space="PSUM"))
            ps_o = ex.enter_context(tc.tile_pool(name="pso", bufs=1, space="PSUM"))
            ps_h = ex.enter_context(tc.tile_pool(name="psh", bufs=1, space="PSUM"))

            const = pool_const.tile([128, 631], f32)
            nc.sync.dma_start(const[:], bcasts)
            wc = pool_const.tile([100, 90], f32)
            nc.sync.dma_start(wc[:], w_cat.rearrange("(s p) c -> p s c", p=100))
            wp = pool_const.tile([100, 200], f32)
            nc.sync.dma_start(wp[:], w_pred)

            ident = const[:, 0:128]
            c0b = const[:, 128:228]
            c1b = const[:, 228:328]
            owb = const[:, 328:428]
            sbb = const[:, 428:528]
            dbb = const[:, 528:628]
            outbc = const[:, 628:629]
            one_b = const[:, 629:630]
            erfb = const[:, 630:631]

            imp = pool_acc.tile([128, 9], f32)
            nc.vector.memset(imp[:], 0.0)
            lod = pool_acc.tile([128, 9], f32)
            nc.vector.memset(lod[:], 0.0)
            h_all = pool_acc.tile([128, NT * 100], f32)

            def v9(ap):  # [128, JE] -> [128, G, 9]
                return ap.rearrange("p (j e) -> p j e", e=9)

            def jb(ap):  # [128, G] -> [128, G, 9] broadcast
                return ap.rearrange("p (j o) -> p j o", o=1).to_broadcast(
                    (128, G, 9)
                )

            for g in range(NG):
                r0, r1 = g * G * 128, (g + 1) * G * 128
                tmem = pool_in.tile([128, G * 100], f32, tag="tmem")
                nc.sync.dma_start(
                    tmem[:], x_mem[r0:r1].rearrange("(j p) d -> p j d", p=128)
                )
                trec = pool_in.tile([128, G * 400], f32, tag="trec")
                nc.sync.dma_start(
                    trec[:], x_rec[r0:r1].rearrange("(j p) d -> p j d", p=128)
                )
                tspa = pool_in.tile([128, G * 400], f32, tag="tspa")
                nc.sync.dma_start(
                    tspa[:], x_spa[r0:r1].rearrange("(j p) d -> p j d", p=128)
                )
                tnod = pool_in.tile([128, G * 100], f32, tag="tnod")
                nc.sync.dma_start(
                    tnod[:], x_node[r0:r1].rearrange("(j p) d -> p j d", p=128)
                )
                tnoi = pool_in.tile([128, G * 9], f32, tag="tnoi")
                nc.sync.dma_start(
                    tnoi[:], x_noi[r0:r1].rearrange("(j p) d -> p j d", p=128)
                )
                tmsc = pool_in.tile([128, G * 2], f32, tag="tmsc")
                nc.sync.dma_start(
                    tmsc[:], x_msc[r0:r1].rearrange("(j p) d -> p j d", p=128)
                )

                nc.scalar.activation(trec[:], trec[:], Act.Relu)
                nc.scalar.activation(tmem[:], tmem[:], Act.Relu)

                pg = ps_g.tile([128, G * 18], f32)

                for j in range(G):
                    t = g * G + j
                    mem = tmem[:, j * 100 : (j + 1) * 100]
                    rec4 = trec[:, j * 400 : (j + 1) * 400]
                    spa4 = tspa[:, j * 400 : (j + 1) * 400]
                    node = tnod[:, j * 100 : (j + 1) * 100]
                    ldg = tmsc[:, j * 2 : j * 2 + 1]

                    fac = pool_f.tile([128, 100], f32, tag="fac")
                    nc.vector.scalar_tensor_tensor(
                        fac[:], c1b, ldg, c0b, Alu.mult, Alu.add
                    )
                    # spa *= factor (broadcast over the 4 experts), in place
                    sp3 = spa4.rearrange("p (e d) -> p e d", e=4)
                    facb = fac[:].rearrange("p (o d) -> p o d", o=1).to_broadcast(
                        (128, 4, 100)
                    )
                    nc.gpsimd.tensor_tensor(sp3, sp3, facb, Alu.mult)

                    psa = ps_t.tile([128, 640], f32)
                    nc.tensor.matmul(
                        psa[0:100, 0:128], mem, ident,
                        is_transpose=True, start=True, stop=True,
                        skip_group_check=True,
                    )
                    for e in range(4):
                        nc.tensor.matmul(
                            psa[0:100, 128:256],
                            spa4[:, e * 100 : (e + 1) * 100], ident,
                            is_transpose=True, start=(e == 0), stop=(e == 3),
                            skip_group_check=True,
                        )
                    for e in range(4):
                        nc.tensor.matmul(
                            psa[0:100, 256:384],
                            rec4[:, e * 100 : (e + 1) * 100], ident,
                            is_transpose=True, start=(e == 0), stop=(e == 3),
                            skip_group_check=True,
                        )
                    nc.tensor.matmul(
                        psa[0:100, 512:640], node, ident,
                        is_transpose=True, start=True, stop=True,
                        skip_group_check=True,
                    )
                    catT = pool_ct.tile([100, 640], f32)
                    nc.scalar.copy(catT[:, 0:384], psa[0:100, 0:384])
                    nc.scalar.copy(catT[:, 512:640], psa[0:100, 512:640])
                    # x2'^T = mem^T * S^T * R^T, written straight into catT
                    x2t = pool_f.tile([100, 128], f32, tag="x2t")
                    nc.vector.tensor_mul(x2t[:], catT[:, 0:128], catT[:, 128:256])
                    nc.vector.tensor_mul(catT[:, 384:512], x2t[:], catT[:, 256:384])
                    for s in range(5):
                        nc.tensor.matmul(
                            pg[:, j * 18 : (j + 1) * 18],
                            catT[:, s * 128 : (s + 1) * 128],
                            wc[:, s * 18 : (s + 1) * 18],
                            start=(s == 0),
                            stop=(s == 4),
                            skip_group_check=True,
                        )

                # ---------- routing (batched over the group) ----------
                pg3 = pg[:].rearrange("p (j c) -> p j c", c=18)
                clean = pg3[:, :, 0:9]
                nzl = pg3[:, :, 9:18]

                stdt = pool_rt.tile([128, JE], f32, tag="stdt")
                nc.scalar.activation(v9(stdt[:]), nzl, Act.Exp)
                nc.scalar.activation(stdt[:], stdt[:], Act.Ln, bias=one_b)
                nc.vector.tensor_scalar_add(stdt[:], stdt[:], 0.01)

                nzy = pool_rt.tile([128, JE], f32, tag="nzy")
                nc.vector.tensor_mul(nzy[:], tnoi[:], stdt[:])
                nc.vector.tensor_tensor(v9(nzy[:]), v9(nzy[:]), clean, Alu.add)

                m1 = pool_rt.tile([128, G], f32, tag="m1")
                nc.vector.reduce_max(m1[:], v9(nzy[:]), axis=AX)
                mk1 = pool_rt.tile([128, JE], f32, tag="mk1")
                nc.vector.tensor_tensor(
                    v9(mk1[:]), v9(nzy[:]), jb(m1[:]), Alu.is_equal
                )
                ny2 = pool_rt.tile([128, JE], f32, tag="ny2")
                nc.vector.scalar_tensor_tensor(
                    ny2[:], mk1[:], -1e30, nzy[:], Alu.mult, Alu.add
                )
                m2 = pool_rt.tile([128, G], f32, tag="m2")
                nc.vector.reduce_max(m2[:], v9(ny2[:]), axis=AX)
                mk2 = pool_rt.tile([128, JE], f32, tag="mk2")
                nc.vector.tensor_tensor(
                    v9(mk2[:]), v9(ny2[:]), jb(m2[:]), Alu.is_equal
                )
                ny3 = pool_rt.tile([128, JE], f32, tag="ny3")
                nc.vector.scalar_tensor_tensor(
                    ny3[:], mk2[:], -1e30, ny2[:], Alu.mult, Alu.add
                )
                m3 = pool_rt.tile([128, G], f32, tag="m3")
                nc.vector.reduce_max(m3[:], v9(ny3[:]), axis=AX)

                dl = pool_rt.tile([128, G], f32, tag="dl")
                nc.vector.tensor_sub(dl[:], m1[:], m2[:])
                ed = pool_rt.tile([128, G], f32, tag="ed")
                nc.scalar.activation(ed[:], dl[:], Act.Exp, scale=-1.0)
                nc.vector.tensor_scalar_add(ed[:], ed[:], 1.0)
                g1 = pool_rt.tile([128, G], f32, tag="g1")
                nc.vector.reciprocal(g1[:], ed[:])
                g2 = pool_rt.tile([128, G], f32, tag="g2")
                nc.vector.tensor_scalar(g2[:], g1[:], -1.0, 1.0, Alu.mult, Alu.add)

                maskb = (
                    tmsc[:]
                    .rearrange("p (j c) -> p j c", c=2)[:, :, 1:2]
                    .to_broadcast((128, G, 9))
                )
                gt = pool_rt.tile([128, JE], f32, tag="gt")
                nc.vector.tensor_tensor(v9(gt[:]), v9(mk1[:]), jb(g1[:]), Alu.mult)
                gt2 = pool_rt.tile([128, JE], f32, tag="gt2")
                nc.vector.tensor_tensor(v9(gt2[:]), v9(mk2[:]), jb(g2[:]), Alu.mult)
                nc.vector.tensor_add(gt[:], gt[:], gt2[:])
                nc.vector.tensor_tensor(v9(gt[:]), v9(gt[:]), maskb, Alu.mult)

                t9 = pool_rt.tile([128, 9], f32, tag="t9")
                nc.vector.reduce_sum(
                    t9[:], gt[:].rearrange("p (j e) -> p e j", e=9), axis=AX
                )
                nc.vector.tensor_add(imp[:], imp[:], t9[:])

                isin = pool_rt.tile([128, JE], f32, tag="isin")
                nc.vector.tensor_tensor(
                    v9(isin[:]), v9(nzy[:]), jb(m3[:]), Alu.is_gt
                )
                d32 = pool_rt.tile([128, G], f32, tag="d32")
                nc.vector.tensor_sub(d32[:], m3[:], m2[:])
                thr = pool_rt.tile([128, JE], f32, tag="thr")
                nc.vector.tensor_tensor(v9(thr[:]), v9(isin[:]), jb(d32[:]), Alu.mult)
                nc.vector.tensor_tensor(v9(thr[:]), v9(thr[:]), jb(m2[:]), Alu.add)
                zz = pool_rt.tile([128, JE], f32, tag="zz")
                nc.vector.tensor_tensor(v9(zz[:]), clean, v9(thr[:]), Alu.subtract)
                rsd = pool_rt.tile([128, JE], f32, tag="rsd")
                nc.scalar.activation(rsd[:], stdt[:], Act.Ln)
                nc.scalar.activation(rsd[:], rsd[:], Act.Exp, scale=-1.0)
                nc.vector.tensor_mul(zz[:], zz[:], rsd[:])
                nc.scalar.activation(
                    zz[:], zz[:], Act.Erf,
                    scale=0.7071067811865476, bias=erfb,
                )
                nc.vector.tensor_scalar(zz[:], zz[:], 0.5, 0.5, Alu.mult, Alu.add)
                nc.vector.tensor_tensor(v9(zz[:]), v9(zz[:]), maskb, Alu.mult)
                t9b = pool_rt.tile([128, 9], f32, tag="t9b")
                nc.vector.reduce_sum(
                    t9b[:], zz[:].rearrange("p (j e) -> p e j", e=9), axis=AX
                )
                nc.vector.tensor_add(lod[:], lod[:], t9b[:])

                # ---------- combine + predictor ----------
                for j in range(G):
                    t = g * G + j
                    mem = tmem[:, j * 100 : (j + 1) * 100]
                    rec4 = trec[:, j * 400 : (j + 1) * 400]
                    spa4 = tspa[:, j * 400 : (j + 1) * 400]  # already *factor
                    tall = pool_b.tile([128, 900], f32, tag="tall")
                    for e in range(9):
                        gcol = gt[:, 9 * j + e : 9 * j + e + 1]
                        if e == 0:
                            src = mem
                        elif e < 5:
                            src = spa4[:, (e - 1) * 100 : e * 100]
                        else:
                            src = rec4[:, (e - 5) * 100 : (e - 4) * 100]
                        eng = nc.vector if e < 4 else nc.gpsimd
                        eng.tensor_scalar(
                            tall[:, e * 100 : (e + 1) * 100], src, gcol, None,
                            Alu.mult,
                        )
                    comb = pool_b.tile([128, 100], f32, tag="comb")
                    nc.vector.reduce_sum(
                        comb[:], tall[:].rearrange("p (e d) -> p d e", e=9),
                        axis=AX,
                    )
                    pso = ps_o.tile([128, 128], f32)
                    nc.tensor.matmul(
                        pso[0:100, 0:128], comb[:], ident,
                        is_transpose=True, start=True, stop=True,
                        skip_group_check=True,
                    )
                    outT = pool_b.tile([100, 128], f32, tag="outT")
                    nc.scalar.copy(outT[:], pso[0:100, :])
                    ph = ps_h.tile([128, 100], f32)
                    c = 0 if t < T3 else 1
                    nc.tensor.matmul(
                        ph[:], outT[:], wp[:, c * 100 : (c + 1) * 100],
                        start=True, stop=True, skip_group_check=True,
                    )
                    bias = sbb if t < T3 else dbb
                    nc.vector.tensor_add(
                        h_all[:, t * 100 : (t + 1) * 100], ph[:], bias
                    )

            # ---------- edge predictor pairing ----------
            W3 = T3 * 100
            o_pn_t = pool_acc.tile([128, 2 * T3], f32)
            owb3 = owb.rearrange("p (o d) -> p o d", o=1).to_broadcast(
                (128, T3, 100)
            )
            for half in range(2):
                off = (1 + half) * W3
                u = pool_pair.tile([128, W3], f32, tag="u")
                nc.vector.tensor_add(u[:], h_all[:, 0:W3], h_all[:, off : off + W3])
                nc.scalar.activation(u[:], u[:], Act.Relu)
                uv = u[:].rearrange("p (t d) -> p t d", d=100)
                nc.vector.tensor_tensor(uv, uv, owb3, Alu.mult)
                pc = pool_pair.tile([128, T3], f32, tag="pc")
                nc.vector.reduce_sum(pc[:], uv, axis=AX)
                nc.vector.tensor_scalar(
                    o_pn_t[:, half * T3 : (half + 1) * T3], pc[:], outbc, None,
                    Alu.add,
                )
            nc.sync.dma_start(o_pn, o_pn_t[:])

            stats_t = pool_acc.tile([128, 18], f32)
            nc.vector.tensor_copy(stats_t[:, 0:9], imp[:])
            nc.vector.tensor_copy(stats_t[:, 9:18], lod[:])
            nc.sync.dma_start(o_stats, stats_t[:])

    return body


def _host_prep(inputs):
    """Fold weights, shard + pad per core. Returns (ins_list, output_like)."""
    f32 = np.float32
    mem = np.ascontiguousarray(inputs["memory_feats"], f32)
    rec = np.ascontiguousarray(inputs["recent_raw"], f32).reshape(B_FULL, 400)
    spa = np.ascontiguousarray(inputs["spatial_raw"], f32).reshape(B_FULL, 400)
    node = np.ascontiguousarray(inputs["node_feats_src"], f32)
    noise = np.ascontiguousarray(inputs["noise"], f32)
    deg = inputs["degree"].astype(f32)
    logdeg = np.log(deg + 1.0).astype(f32)

    wg = np.asarray(inputs["w_gate"], f32)
    wn = np.asarray(inputs["w_noise"], f32)

    def fold(w):
        W1, W2, W3_, W4, W5, W6 = (w[i * 100 : (i + 1) * 100] for i in range(6))
        return np.concatenate(
            [W1 + W4, (W2 + W4) / 4.0, (W3_ + W4) / 4.0, W5 / 16.0, W6], axis=0
        )

    w_cat = np.concatenate([fold(wg), fold(wn)], axis=1).astype(f32)  # [500,18]
    w_pred = np.concatenate(
        [np.asarray(inputs["src_w"], f32), np.asarray(inputs["dst_w"], f32)],
        axis=1,
    )  # [100, 200]

    c0 = np.asarray(inputs["deg_coef"], f32)[0, :, 0]
    c1 = np.asarray(inputs["deg_coef"], f32)[0, :, 1]
    ow = np.asarray(inputs["out_w"], f32)[:, 0]
    sb = np.asarray(inputs["src_b"], f32)
    db = np.asarray(inputs["dst_b"], f32)
    ob = float(np.asarray(inputs["out_b"], f32)[0])
    bcasts = np.zeros((128, 631), f32)
    bcasts[:, 629] = 1.0
    bcasts[:, 630] = -0.07071067811865476
    bcasts[:, 0:128] = np.eye(128, dtype=f32)
    bcasts[:, 128:228] = c0
    bcasts[:, 228:328] = c1
    bcasts[:, 328:428] = ow
    bcasts[:, 428:528] = sb
    bcasts[:, 528:628] = db
    bcasts[:, 628] = ob

    ins_list = []
    for k in range(M_CORES):
        xm = np.zeros((BC, 100), f32)
        xr = np.zeros((BC, 400), f32)
        xs = np.zeros((BC, 400), f32)
        xn = np.zeros((BC, 100), f32)
        xz = np.zeros((BC, 9), f32)
        xc = np.zeros((BC, 2), f32)
        for third in range(3):
            lo = third * NUM_EDGE + k * THIRD
            hi = min(lo + THIRD, (third + 1) * NUM_EDGE)
            n = hi - lo
            if n <= 0:
                continue
            dst = slice(third * THIRD, third * THIRD + n)
            xm[dst] = mem[lo:hi]
            xr[dst] = rec[lo:hi]
            xs[dst] = spa[lo:hi]
            xn[dst] = node[lo:hi]
            xz[dst] = noise[lo:hi]
            xc[dst, 0] = logdeg[lo:hi]
            xc[dst, 1] = 1.0
        ins_list.append((xm, xr, xs, xn, xz, xc, w_cat, w_pred, bcasts))

    output_like = (
        np.zeros((128, 2 * T3), f32),
        np.zeros((128, 18), f32),
    )
    return ins_list, output_like


def kernel(**inputs):
    assert int(inputs["neg_samples"]) == 1
    from concourse import tile
    from concourse.bass_test_utils import run_kernel

    ins_list, output_like = _host_prep(inputs)
    body = _make_body(BC, G)
    res = run_kernel(
        body,
        None,
        ins_list,
        output_like=[output_like] * M_CORES,
        bass_type=tile.TileContext,
        check_with_sim=False,
        check_with_hw=True,
        num_cores=M_CORES,
    )
    assert res is not None

    pos = np.zeros((NUM_EDGE, 1), np.float32)
    neg = np.zeros((NUM_EDGE, 1), np.float32)
    imp = np.zeros(9, np.float64)
    lod = np.zeros(9, np.float64)
    for k in range(M_CORES):
        rk = res.results[k]
        o_pn = next(v for v in rk.values() if v.shape == (128, 2 * T3))
        o_st = next(v for v in rk.values() if v.shape == (128, 18))
        n = min(THIRD, NUM_EDGE - k * THIRD)
        if n > 0:
            # row (t*128+p) of this core's third -> o_pn[p, t]
            posk = o_pn[:, 0:T3].T.reshape(-1)[:n]   # [t, p] -> flat
            negk = o_pn[:, T3 : 2 * T3].T.reshape(-1)[:n]
            pos[k * THIRD : k * THIRD + n, 0] = posk
            neg[k * THIRD : k * THIRD + n, 0] = negk
        imp += o_st[:, 0:9].sum(axis=0, dtype=np.float64)
        lod += o_st[:, 9:18].sum(axis=0, dtype=np.float64)

    def cv2(x):
        x = x.astype(np.float32)
        return np.var(x, ddof=1) / (np.mean(x) ** 2 + 1e-10)

    loss = np.float32(0.4 * (cv2(imp) + cv2(lod)))
    if getattr(res, "exec_time_ns", None):
        print(f"HW exec time: {res.exec_time_ns} ns")
    return pos, neg, loss
